# revision 53
# baseline (speedup 1.0000x reference)
"""Causal self-attention (B=2, T=2048, C=768, H=12) on 8 TRN2 NeuronCores.

Sharding: core i handles batch b = i//4 and 3 consecutive heads h0 = 3*(i%4).
Each core produces a partial projection output [T, C] (sum over its 3 heads);
the host sums the 4 partials per batch and adds biases.

Per-core dataflow (all transposeless):
  - QK gen:  psum[128,512] = sum_ct Wqk[ct,h].T @ xT[ct]  -> rows 0:64 = Q^T
             (scale+bias folded), rows 64:128 = K^T.
  - V gen:   psum[128,192] = sum_ct xT[ct,tchunk].T @ Wv[ct] -> v in natural
             [T, hs] layout, stored per k-tile as [v | 1] (ones col -> denom).
  - Attn:    S^T tile = K_block @ Q^T  ([128 kpos, 512 q] in PSUM); causal
             mask applied by accumulating -60000*tril into the diagonal
             128-block via a second matmul (I stationary, negtri moving);
             exp on ACT (no max subtraction; scores are O(1)),
             PV accumulates [y^T | denom] over k-tiles.
  - Norm:    raw denominator row broadcast across partitions via a K=1
             matmul, reciprocal_approx_fast on DVE (51 ULP, no activation
             table switches), y^T = y_unnorm^T * bcast; h1's result is
             relocated to partitions 64:128 by a small SBUF DMA.
  - Proj:    po[tchunk, :] = [y_h0; y_h1]^T @ [Wp_h0; Wp_h1] (K=128 all
             real) + y_h2^T @ Wp_h2, bf16 out (PSUM -> SBUF -> DRAM).

Schedule: attention (jq, h) units run as 2-lane round-robined pairs with
one k-tile per lane per round (double-buffered S psum so S never waits on
exp); QKV generation, normalize, projection and output DMA drip into the
pairs' rounds as PE filler so exp (ACT) latency is hidden and the PE never
idles long enough for the HAM clock governor to re-throttle 2.4 -> 1.2 GHz.
Inputs ride the two hardware DGE queues (gpsimd/scalar) as few large
transfers ordered by consumption time.
"""

import os

os.environ.setdefault("MYCRO_LOCAL_CACHE", "1")

import numpy as np
import ml_dtypes

BF16_NP = ml_dtypes.bfloat16

import concourse.bass as bass
import concourse.bacc as bacc
import concourse.mybir as mybir
import concourse.tile as tile
from concourse.bass_utils import run_bass_kernel_spmd

T = 2048
C = 768
HS = 64
NH = 12
HPC = 3  # heads per core
NCORES = 8
CT = C // 128  # 6 contraction tiles for qkv/v gen
QC = 512  # q-chunk width
NQC = T // QC  # 4
NKT = T // 128  # 16 k-tiles (and T-chunks)
SCALE = 1.0 / 8.0  # 1/sqrt(HS)
F32 = mybir.dt.float32
BF16 = mybir.dt.bfloat16

_PROGRAM = None


def _build_program():
    nc = bacc.Bacc("TRN2")
    xT_d = nc.declare_dram_parameter("xT", [128, CT, T], BF16, isOutput=False)
    wqk_d = nc.declare_dram_parameter("wqk", [128, CT, HPC, 128], BF16, isOutput=False)
    wv_d = nc.declare_dram_parameter("wv", [128, CT, HPC * HS], BF16, isOutput=False)
    wp_d = nc.declare_dram_parameter("wp", [128, 2, C], BF16, isOutput=False)
    bq_d = nc.declare_dram_parameter("bq", [HS, HPC], F32, isOutput=False)
    ident_d = nc.declare_dram_parameter("ident", [128, 128], BF16, isOutput=False)
    negtri_d = nc.declare_dram_parameter("negtri", [128, 2, 128], BF16, isOutput=False)
    out_d = nc.declare_dram_parameter("out", [T, C], BF16, isOutput=True)

    with tile.TileContext(nc) as tc:
        with (
            tc.tile_pool(name="const", bufs=1) as constp,
            tc.tile_pool(name="big", bufs=1) as bigp,
            tc.tile_pool(name="exps", bufs=8) as expp,
            tc.tile_pool(name="work", bufs=3) as workp,
            tc.tile_pool(name="ps_s", bufs=2, space="PSUM") as ps_s,
            tc.tile_pool(name="ps_y", bufs=1, space="PSUM") as ps_y,
            tc.tile_pool(name="ps_m", bufs=2, space="PSUM") as ps_m,
        ):
            # ---- clock-governor warm-up: the HAM governor needs ~3us of
            # continuous PE busy to lift the clock 1.2 -> 2.4 GHz; burn
            # matmuls on memset tiles while the input DMA lands.
            warm_a = constp.tile([128, 128], BF16, name="warm_a")
            warm_b = constp.tile([128, QC], BF16, name="warm_b")
            nc.vector.memset(warm_a, 0.125)
            nc.vector.memset(warm_b, 0.125)
            for i in range(14):
                pw = ps_m.tile([128, QC], F32, tag="misc", name=f"warm{i}")
                nc.tensor.matmul(pw, warm_a, warm_b, start=True, stop=True)

            # ---- inputs -> SBUF; priority order on the sync queue: wqk + bq
            # + mask consts + the first 512-col chunk of every xT c-tile
            # (feeds qkgen(0) jq0) + wv (vgen); later xT chunks + wp go via
            # the (otherwise idle) gpsimd engine's DGE so their triggers
            # don't occupy ACT.
            # DMA triggers cost ~0.6us each on their engine queue, and the
            # sync queue is the slow software-dynamic path -- so all input
            # loads go on the two hardware DGE queues (gpsimd + scalar),
            # few large transfers, split into halves for rolling
            # consumption, ordered by consumption time. sync only carries
            # the (late) output stores.
            wqk = constp.tile([128, CT, HPC, 128], BF16)
            wv = constp.tile([128, CT, HPC * HS], BF16)
            xT = bigp.tile([128, CT, T], BF16)
            bq = constp.tile([HS, HPC], F32)
            ident = constp.tile([128, 128], BF16)
            negtri2 = constp.tile([128, 2, 128], BF16)
            wp = constp.tile([128, 2, C], BF16)
            h3 = CT // 2
            nc.gpsimd.dma_start(out=wqk, in_=wqk_d[:])
            nc.gpsimd.dma_start(out=xT[:, 0:h3, 0:QC], in_=xT_d[:, 0:h3, 0:QC])
            nc.gpsimd.dma_start(
                out=xT[:, 0:h3, QC : 2 * QC], in_=xT_d[:, 0:h3, QC : 2 * QC]
            )
            nc.gpsimd.dma_start(
                out=xT[:, 0:h3, 2 * QC : T], in_=xT_d[:, 0:h3, 2 * QC : T]
            )
            nc.scalar.dma_start(out=bq, in_=bq_d[:])
            nc.scalar.dma_start(out=xT[:, h3:CT, 0:QC], in_=xT_d[:, h3:CT, 0:QC])
            nc.scalar.dma_start(out=ident, in_=ident_d[:])
            nc.scalar.dma_start(out=negtri2, in_=negtri_d[:])
            nc.scalar.dma_start(out=wv, in_=wv_d[:])
            nc.scalar.dma_start(
                out=xT[:, h3:CT, QC : 2 * QC], in_=xT_d[:, h3:CT, QC : 2 * QC]
            )
            nc.scalar.dma_start(
                out=xT[:, h3:CT, 2 * QC : T], in_=xT_d[:, h3:CT, 2 * QC : T]
            )
            # wp slot 0 = [wp_h0; wp_h1] stacked on partitions (K=128 all
            # real), slot 1 = [wp_h2; zeros]
            nc.scalar.dma_start(out=wp, in_=wp_d[:])

            ones_f = constp.tile([128, HS], F32)
            nc.vector.memset(ones_f, 1.0)

            qT = bigp.tile([128, HPC, T], BF16)
            kT = bigp.tile([128, HPC, T], BF16)
            vsb = bigp.tile([128, NKT, HPC, HS + 1], BF16)  # [v | 1] per head
            yT01 = bigp.tile([128, T], BF16)  # rows 0:64 h0, 64:128 h1
            yT2 = bigp.tile([128, T], BF16)  # rows 0:64 h2, 64:128 zero
            yh1s = bigp.tile([HS, T], BF16)  # h1 normalize staging
            yun = bigp.tile([128, HPC, T], F32)  # unnorm y^T; row 64 = denom

            nc.gpsimd.memset(vsb[:, :, :, HS], 1.0)
            # zero the bottom halves: K=64 contractions are padded to K=128
            # (zero rows are numerically free) because half-array row-group
            # matmuls do not count as PE-busy for the HAM clock governor --
            # with K=64 the whole attention phase runs at 1.2 GHz (K=4/8).
            # pair 1 only touches the jq-0 slices of heads 0-1: zero just
            # those up front (0.3us) so the 3us bulk DVE memset doesn't sit
            # ahead of the kT/qT evacuations the first S matmuls wait on
            nc.vector.memset(qT[HS:128, 0:2, 0:QC], 0.0)
            nc.gpsimd.memset(kT[HS:128, :, :], 0.0)

            # ---- QK^T generation, one (h, jq) psum group at a time so the
            # consumption order tracks the x-chunk DMA arrival order
            def qk_group(h, jq):
                pqk = ps_m.tile([128, QC], F32, tag="misc", name=f"pqk{h}_{jq}")
                for ct in range(CT):
                    nc.tensor.matmul(
                        pqk,
                        wqk[:, ct, h, :],
                        xT[:, ct, jq * QC : (jq + 1) * QC],
                        start=(ct == 0),
                        stop=(ct == CT - 1),
                    )
                nc.vector.tensor_scalar_add(
                    qT[0:HS, h, jq * QC : (jq + 1) * QC],
                    pqk[0:HS, :],
                    bq[:, h : h + 1],
                )
                # partition-shifting evacuation (64:128 -> 0:64); legal on
                # DVE at 64 channels (bank0->Q0, bank1->Q1, reads follow
                # the src access pattern)
                nc.vector.tensor_copy(
                    kT[0:HS, h, jq * QC : (jq + 1) * QC], pqk[64:128, :]
                )

            # ---- V generation (natural [T, hs] layout, + ones column)
            def vgen_m(m):
                pv = ps_m.tile([128, QC], F32, tag="misc", name=f"pv{m}")
                for ct in range(CT):
                    nc.tensor.matmul(
                        pv[:, 0 : HPC * HS],
                        xT[:, ct, m * 128 : (m + 1) * 128],
                        wv[:, ct, :],
                        start=(ct == 0),
                        stop=(ct == CT - 1),
                    )
                nc.vector.tensor_copy(
                    vsb[:, m, :, 0:HS],
                    pv[:, 0 : HPC * HS].rearrange("p (h d) -> p h d", h=HPC),
                )

            # chunk-major minimal prologue: only what attention pair 1
            # (q-chunk 0, heads 0-1) needs; everything else drips into the
            # attention pairs as PE filler work.
            qk_group(0, 0)
            qk_group(1, 0)
            # bulk of the qT zero-fill, now behind the first evacuations
            nc.vector.memset(qT[HS:128, 0:2, QC:T], 0.0)
            nc.vector.memset(qT[HS:128, 2, :], 0.0)
            for m in range(4):
                vgen_m(m)
            nc.gpsimd.memset(yT2[HS:128, :], 0.0)

            def warm(n, w=QC):
                # HAM insurance: dependency-free matmuls the PE can chew on
                # while a DMA chunk lands or a normalize chain propagates,
                # so a >1us idle gap never re-throttles the clock
                def f():
                    for i in range(n):
                        pw = ps_m.tile([128, QC], F32, tag="misc")
                        nc.tensor.matmul(
                            pw[:, 0:w], warm_a, warm_b[:, 0:w], start=True, stop=True
                        )

                return f

            # ---- attention: interleave pairs of independent (jq, h)
            # units so the PE streams one unit's matmuls while ACT runs the
            # other's exp (keeps the PE dense -> HAM stays at 2.4 GHz)

            def tile_geom(jq, kt):
                if kt < 4 * jq:  # full k-tile
                    return QC, 0
                r = kt - 4 * jq
                return QC - 128 * r, 128 * r

            def s_mms(es_p, li, jq, h, kt):
                q0 = jq * QC
                w, qoff = tile_geom(jq, kt)
                nc.tensor.matmul(
                    es_p[:, li, 0:w],
                    kT[:, h, kt * 128 : (kt + 1) * 128],
                    qT[:, h, q0 + qoff : q0 + QC],
                    start=True,
                    stop=True,
                )
                if kt >= 4 * jq:
                    # diagonal 128-block: add -60000 * tril to the S psum
                    # (exp of masked entries underflows to exactly 0), in
                    # place of a DVE mask-multiply after the exp
                    nc.tensor.matmul(
                        es_p[:, li, 0:128],
                        ident,
                        negtri2[:, 0, :],
                        start=False,
                        stop=True,
                        skip_group_check=True,
                    )

            def exp_tiles(es_p, es_b, lanes_g):
                # one ACTIVATE covering every active lane (3D AP over the
                # lane dim) when the lanes share a span width, else per-lane
                ws = [tile_geom(ln["jq"], ln["_g"])[0] for ln in lanes_g]
                if len(lanes_g) == 2 and ws[0] == ws[1]:
                    nc.scalar.activation(
                        es_b[:, :, 0 : ws[0]],
                        es_p[:, :, 0 : ws[0]],
                        mybir.ActivationFunctionType.Exp,
                    )
                else:
                    for ln, w in zip(lanes_g, ws):
                        nc.scalar.activation(
                            es_b[:, ln["li"], 0:w],
                            es_p[:, ln["li"], 0:w],
                            mybir.ActivationFunctionType.Exp,
                        )

            def pv_mms(py, es_b, li, jq, h, kt):
                w, qoff = tile_geom(jq, kt)
                nc.tensor.matmul(
                    py[0 : HS + 1, qoff:QC],
                    vsb[:, kt, h, :],
                    es_b[:, li, 0:w],
                    start=(kt == 0),
                    stop=(kt == 4 * jq + 3),
                    skip_group_check=True,
                )

            def normalize(jq, h):
                # broadcast the raw denominator row across 64 partitions via
                # a K=1 matmul, reciprocal it with one approx DVE op (51 ULP,
                # no ACT involvement, no activation-table switches), multiply.
                # h0 -> yT01 rows 0:64; h2 -> yT2 rows 0:64; h1 stages into
                # yh1s then a small SBUF->SBUF DMA lands it in yT01 rows
                # 64:128 (the partition up-shift DVE can't do) so proj's
                # first matmul contracts h0+h1 with a fully-real K=128.
                q0 = jq * QC
                pb = ps_m.tile([128, QC], F32, tag="misc", name=f"pb{jq}_{h}")
                nc.tensor.matmul(
                    pb[0:HS, :],
                    ones_f[64:65, 0:HS],
                    yun[64:65, h, q0 : q0 + QC],
                    start=True,
                    stop=True,
                )
                rb = workp.tile([HS, QC], F32, tag="rb", name=f"rb{jq}_{h}")
                nc.vector.reciprocal_approx_fast(rb, pb[0:HS, :])
                dst = (
                    yT01[0:HS, q0 : q0 + QC]
                    if h == 0
                    else yT2[0:HS, q0 : q0 + QC]
                    if h == 2
                    else yh1s[:, q0 : q0 + QC]
                )
                nc.vector.tensor_mul(dst, yun[0:HS, h, q0 : q0 + QC], rb)
                if h == 1:
                    nc.gpsimd.dma_start(
                        out=yT01[HS:128, q0 : q0 + QC], in_=yh1s[:, q0 : q0 + QC]
                    )

            def proj(t):
                ob = workp.tile([128, C], BF16, tag="ob", name=f"ob{t}")
                for n0, w in ((0, 512), (512, 256)):
                    po = ps_m.tile([128, QC], F32, tag="misc", name=f"po{t}_{n0}")
                    nc.tensor.matmul(
                        po[:, 0:w],
                        yT01[:, t * 128 : (t + 1) * 128],
                        wp[:, 0, n0 : n0 + w],
                        start=True,
                        stop=False,
                    )
                    nc.tensor.matmul(
                        po[:, 0:w],
                        yT2[:, t * 128 : (t + 1) * 128],
                        wp[:, 1, n0 : n0 + w],
                        start=False,
                        stop=True,
                    )
                    nc.vector.tensor_copy(ob[:, n0 : n0 + w], po[:, 0:w])
                # late chunks ride the fast gpsimd DGE; early/slack chunks
                # take the slow sync path so ~11us of trigger issue doesn't
                # serialize on one queue during the final proj burst
                eng = nc.gpsimd if 4 <= t < 12 else nc.sync
                eng.dma_start(out=out_d[t * 128 : (t + 1) * 128, :], in_=ob)

            def stash(ln):
                jq, h, py = ln["jq"], ln["h"], ln["py"]
                q0 = jq * QC
                # stash unnormalized y + denominator row; frees the PSUM slot
                nc.vector.tensor_copy(
                    yun[0 : HS + 1, h, q0 : q0 + QC], py[0 : HS + 1, :]
                )

            def run_lanes(lane_units, fillers=()):
                """Run a round-group of 2 attention units; drip filler thunks
                (gen tail / normalize / proj of completed chunks) between
                rounds so the PE has independent work while ACT runs exp."""
                fillers = list(fillers)
                lanes = []
                for li, (jq, h) in enumerate(lane_units):
                    lanes.append(
                        {
                            "jq": jq,
                            "h": h,
                            "G": 4 * jq + 4,  # one k-tile per round
                            "py": ps_y.tile(
                                [128, QC], F32, tag=f"py{li}", name=f"py{li}_{jq}_{h}"
                            ),
                            "li": li,
                            "ebs": {},
                        }
                    )
                fillers = [f if isinstance(f, tuple) else (0, f) for f in fillers]
                max_g = max(ln["G"] for ln in lanes)
                warm_s = warm(1, 256)
                for g in range(max_g + 1):  # PV lags S by one round
                    elig = [f for f in fillers if f[0] <= g]
                    rounds_left = max_g + 1 - g
                    npop = -(-len(elig) // rounds_left) if elig else 0
                    for f in elig[:npop]:
                        fillers.remove(f)
                        f[1]()
                    lanes_g = [ln for ln in lanes if g < ln["G"]]
                    if npop == 0 and lanes_g:
                        # ACT-bound round (exp ~1.15us vs PE ~0.85us): burn a
                        # small dependency-free matmul so PE micro-idles
                        # don't accumulate into a HAM re-throttle
                        warm_s()
                    if lanes_g:
                        # double-buffered (bufs=2) so round g+1's S matmuls
                        # never wait on round g's exp
                        es_p = ps_s.tile(
                            [128, 2, QC],
                            F32,
                            tag="es",
                            name=f"es_{lanes[0]['jq']}_{lanes[0]['h']}_{g}",
                        )
                        es_b = expp.tile([128, 2, QC], BF16, tag="ex")
                        for ln in lanes_g:
                            ln["_g"] = g
                            ln["ebs"][g] = es_b
                            s_mms(es_p, ln["li"], ln["jq"], ln["h"], g)
                        exp_tiles(es_p, es_b, lanes_g)
                    for ln in lanes:
                        if 0 <= g - 1 < ln["G"]:
                            pv_mms(
                                ln["py"],
                                ln["ebs"].pop(g - 1),
                                ln["li"],
                                ln["jq"],
                                ln["h"],
                                g - 1,
                            )
                        if g == ln["G"]:  # early stash frees py + unblocks
                            stash(ln)  # this unit's normalize as a filler
                for _, f in fillers:
                    f()

            def N(jq, h):
                return lambda: normalize(jq, h)

            def P(t):
                return lambda: proj(t)

            # pair schedule: ascending jq so attention starts as soon as
            # x-chunk 0 lands; the rest of QKV-gen drips in as filler under
            # the early pairs' exp; completed chunks' normalize + projection
            # (+ output DMA) fill the late pairs.
            run_lanes([(0, 0), (0, 1)])
            warm(3)()
            qk_group(0, 1)
            warm(2)()
            qk_group(1, 1)
            warm(2)()
            for m in range(4, 8):
                vgen_m(m)
                warm(1)()
            # chunk-2/3-dependent fillers carry a min-round so they don't
            # enter the in-order PE queue before their DMA can have landed
            run_lanes(
                [(1, 0), (1, 1)],
                [(0, warm(2)), (1, warm(2)), (2, warm(1))]
                + [(3, lambda: qk_group(0, 2)), (5, lambda: qk_group(1, 2))]
                + [(6 + m - 8, lambda m=m: vgen_m(m)) for m in range(8, 12)],
            )
            run_lanes(
                [(2, 0), (2, 1)],
                [lambda: qk_group(0, 3), warm(2), lambda: qk_group(1, 3), warm(2)]
                + [(lambda m=m: vgen_m(m)) for m in range(12, NKT)],
            )
            run_lanes(
                [(3, 0), (3, 1)],
                [(lambda jq=jq: qk_group(2, jq)) for jq in range(NQC)]
                + [N(0, 0), N(0, 1), N(1, 0), N(1, 1)],
            )
            # pairing (3,2) with (1,2) lets q-chunk 1's normalize + proj ride
            # as late fillers here (norm(1,0/1) done above, (1,2) stashes at
            # round 8), closing this pair's ACT-PE deficit with real work
            run_lanes(
                [(3, 2), (1, 2)],
                [N(2, 0), N(2, 1), N(3, 0), N(3, 1)]
                + [(9, N(1, 2)), (11, P(4)), (12, P(5)), (13, P(6)), (14, P(7))],
            )
            run_lanes(
                [(2, 2), (0, 2)],
                [(0, N(3, 2)), (2, P(12)), (3, P(13)), (4, P(14)), (5, P(15))]
                + [(6, N(0, 2)), (8, P(0)), (9, P(1)), (10, P(2)), (11, P(3))],
            )

            # ---- tail: normalize + proj for q-chunk 2
            fin = [N(2, 2), warm(4), P(8), P(9), P(10), P(11)]
            for f in fin:
                f()
    return nc


def get_program():
    global _PROGRAM
    if _PROGRAM is None:
        _PROGRAM = _build_program()
        if not _PROGRAM.is_finalized():
            _PROGRAM.finalize()
    return _PROGRAM


def make_in_maps(x, W_attn, b_attn):
    x = np.asarray(x, dtype=np.float32)
    W_attn = np.asarray(W_attn, dtype=np.float32)
    b_attn = np.asarray(b_attn, dtype=np.float32)
    ident_arr = np.eye(128, dtype=BF16_NP)
    negtri1 = (
        -60000.0
        * (np.arange(128, dtype=np.int64)[:, None] > np.arange(128, dtype=np.int64)[None, :])
    ).astype(BF16_NP)
    negtri_arr = np.ascontiguousarray(np.stack([negtri1, negtri1], axis=1))
    in_maps = []
    for i in range(NCORES):
        b = i // 4
        h0 = HPC * (i % 4)
        xb = x[b]  # [T, C]
        xT_arr = np.ascontiguousarray(
            xb.T.reshape(CT, 128, T).transpose(1, 0, 2)
        ).astype(BF16_NP)  # [p, ct, t]
        Wq = (
            W_attn[:, h0 * HS : (h0 + HPC) * HS].reshape(C, HPC, HS) * SCALE
        )
        Wk = W_attn[:, C + h0 * HS : C + (h0 + HPC) * HS].reshape(C, HPC, HS)
        wqk_full = np.concatenate([Wq, Wk], axis=2)  # [C, HPC, 128]
        wqk_arr = np.ascontiguousarray(
            wqk_full.reshape(CT, 128, HPC, 128).transpose(1, 0, 2, 3)
        ).astype(BF16_NP)
        wv_arr = np.ascontiguousarray(
            W_attn[:, 2 * C + h0 * HS : 2 * C + (h0 + HPC) * HS]
            .reshape(CT, 128, HPC * HS)
            .transpose(1, 0, 2)
        ).astype(BF16_NP)
        bq_arr = np.ascontiguousarray(
            (b_attn[h0 * HS : (h0 + HPC) * HS] * SCALE).reshape(HPC, HS).T
        )
        in_maps.append(
            {
                "xT": xT_arr,
                "wqk": wqk_arr,
                "wv": wv_arr,
                "bq": bq_arr,
                "ident": ident_arr,
                "negtri": negtri_arr,
            }
        )
    return in_maps


def add_wp(in_maps, W_proj):
    W_proj = np.asarray(W_proj, dtype=np.float32)
    for i in range(NCORES):
        h0 = HPC * (i % 4)
        wph = W_proj[h0 * HS : (h0 + HPC) * HS, :].reshape(HPC, HS, C)
        wp_arr = np.zeros((128, 2, C), dtype=BF16_NP)
        wp_arr[0:HS, 0] = wph[0].astype(BF16_NP)
        wp_arr[HS:128, 0] = wph[1].astype(BF16_NP)
        wp_arr[0:HS, 1] = wph[2].astype(BF16_NP)
        in_maps[i]["wp"] = wp_arr
    return in_maps


def gather(results, b_attn, W_proj, b_proj):
    b_attn = np.asarray(b_attn, dtype=np.float32)
    W_proj = np.asarray(W_proj, dtype=np.float32)
    b_proj = np.asarray(b_proj, dtype=np.float32)
    parts = [np.asarray(r["out"], dtype=np.float32) for r in results]
    out = np.stack(
        [parts[0] + parts[1] + parts[2] + parts[3], parts[4] + parts[5] + parts[6] + parts[7]]
    )
    # b_v adds to y after normalization -> constant vector through the proj.
    # b_k provably cancels in softmax; b_q is handled on-device.
    const = b_proj + b_attn[2 * C : 3 * C] @ W_proj
    return out + const[None, None, :]


def run(x, W_attn, b_attn, W_proj, b_proj, trace=False):
    nc = get_program()
    in_maps = add_wp(make_in_maps(x, W_attn, b_attn), W_proj)
    res = run_bass_kernel_spmd(nc, in_maps, list(range(NCORES)), trace=trace)
    out = gather(res.results, b_attn, W_proj, b_proj)
    return out, res


def kernel(x, W_attn, b_attn, W_proj, b_proj):
    out, _ = run(x, W_attn, b_attn, W_proj, b_proj, trace=False)
    return out
